# revision 1
# baseline (speedup 1.0000x reference)
"""Trainium2 Bass kernel for causal GQA attention with RoPE (dense_transformer).

Problem shapes (hardcoded): x [4, 2048, 2048] f32, Wq [2048,2048], Wk/Wv [2048,1024],
Wo [2048,2048], cos/sin [2048,128]. Output [4, 2048, 2048] f32.

Sharding: 8 cores = 4 batches x 2 query-groups. Core c handles batch b=c//2 and
query blocks {2s+g : s=0..7} (g=c%2) of 128 tokens each. Every core computes the
full K/V projection for its batch (tokens it needs), all 16 heads for its own
query rows, and the full o_proj for those rows. Output rows are disjoint across
cores, so the gather is host-side concatenation - no device collectives.

The per-core program is identical across cores (SPMD); all per-core variation
(which query tokens, RoPE phases, causal masks) is carried in the input data:
 - xqT: gathered+transposed query activations
 - cosq/sinq: RoPE tables gathered for the core's query positions (sign-folded,
   1/sqrt(HD) prescaled)
 - mask: additive causal mask for the last 256 key columns of each slot
Slot s processes keys [0, 256*(s+1)); block-causality makes everything before
that window unmasked and everything after it never computed.

Matmuls run in bf16 (1 cyc/row on the PE vs 4 for fp32) with fp32 PSUM
accumulation. Layouts keep the contraction dim on partitions everywhere:
activations/projections live transposed ([feature, token]); scores are computed
per q-block as [q, key]; softmax (exp without max-subtract - scores are O(1) by
construction) reduces along the free axis; P is PE-transposed to feed P^T as the
moving operand of the AV matmul, which directly yields AttnOut^T, the natural
stationary operand for the final o_proj.
"""

import sys

sys.path.insert(0, "/opt/trn_rl_repo")

import functools
import math
from contextlib import ExitStack

import ml_dtypes
import numpy as np

B, S, H = 4, 2048, 2048
NH, NKV, HD = 16, 8, 128
SQ = S // 2           # query tokens per core
KVD = NKV * HD        # 1024
NSLOT = 8             # q-blocks (slots) per core
NCORES = 8
NEG = -1.0e30
BF16 = ml_dtypes.bfloat16


def _build_program():
    import concourse.mybir as mybir
    import concourse.tile as tile
    from concourse import bacc
    from concourse.masks import make_identity

    dt = mybir.dt
    f32, bf16 = dt.float32, dt.bfloat16
    ADD, MULT = mybir.AluOpType.add, mybir.AluOpType.mult
    EXP = mybir.ActivationFunctionType.Exp
    AXX = mybir.AxisListType.X

    nc = bacc.Bacc("TRN2", target_bir_lowering=False, debug=False)

    xT = nc.dram_tensor("xT", [H, S], bf16, kind="ExternalInput").ap()
    xqT = nc.dram_tensor("xqT", [H, SQ], bf16, kind="ExternalInput").ap()
    wq = nc.dram_tensor("wq", [H, H], bf16, kind="ExternalInput").ap()
    wk = nc.dram_tensor("wk", [H, KVD], bf16, kind="ExternalInput").ap()
    wv = nc.dram_tensor("wv", [H, KVD], bf16, kind="ExternalInput").ap()
    wo = nc.dram_tensor("wo", [H, H], bf16, kind="ExternalInput").ap()
    cosq = nc.dram_tensor("cosq", [HD, SQ], f32, kind="ExternalInput").ap()
    sinq = nc.dram_tensor("sinq", [HD, SQ], f32, kind="ExternalInput").ap()
    cosk = nc.dram_tensor("cosk", [HD, S], f32, kind="ExternalInput").ap()
    sink = nc.dram_tensor("sink", [HD, S], f32, kind="ExternalInput").ap()
    maskT = nc.dram_tensor("maskT", [128, 2, 256], f32, kind="ExternalInput").ap()
    out = nc.dram_tensor("out", [SQ, H], f32, kind="ExternalOutput").ap()
    attnT_dram = nc.dram_tensor("attnT_tmp", [H, SQ], bf16).ap()

    xT_r = xT.rearrange("(a p) t -> p a t", p=128)
    xqT_r = xqT.rearrange("(a p) t -> p a t", p=128)
    wq_r = wq.rearrange("(a p) n -> p a n", p=128)
    wk_r = wk.rearrange("(a p) n -> p a n", p=128)
    wv_r = wv.rearrange("(a p) n -> p a n", p=128)
    wo_r = wo.rearrange("(a p) n -> p a n", p=128)
    attnT_r = attnT_dram.rearrange("(a p) t -> p a t", p=128)

    def rope(pool, ps, cos_sb, sin_sb, dst):
        # dst = ps*cos + rot64(ps)*sin  (sign of the rotation folded into sin).
        # The rotated reads keep ps in PSUM: only PSUM operands may sit at a
        # different start partition than the other operands.
        t1 = pool.tile([128, 512], f32, tag="rope_t1")
        nc.vector.tensor_tensor(t1, ps, cos_sb, MULT)
        t2 = pool.tile([128, 512], f32, tag="rope_t2")
        nc.vector.tensor_tensor(t2[0:64, :], ps[64:128, :], sin_sb[0:64, :], MULT)
        nc.vector.tensor_tensor(t2[64:128, :], ps[0:64, :], sin_sb[64:128, :], MULT)
        nc.gpsimd.tensor_tensor(dst, t1, t2, ADD)

    with tile.TileContext(nc) as tc, ExitStack() as top:
        misc = top.enter_context(tc.tile_pool(name="misc", bufs=1))
        ident = misc.tile([128, 128], bf16)
        make_identity(nc, ident)
        mask_sb = misc.tile([128, 2, 256], f32)  # maskT per kv-block, tiled x2 q
        nc.sync.dma_start(out=mask_sb, in_=maskT)

        kvq = top.enter_context(tc.tile_pool(name="kvq", bufs=1))
        kT_sb = kvq.tile([128, NKV, S], bf16)    # K^T rope'd: [d, kvh, t]
        # V with a ones-column appended per kv head: [tok_p, tok_blk, kvh*129+d];
        # column 128 of each head accumulates the softmax denominator during AV.
        v_sb = kvq.tile([128, S // 128, NKV * (HD + 1)], bf16)
        # Q^T rope'd+scaled, [d, slot, head, qi] so a GQA pair is one
        # contiguous 256-wide moving operand per slot.
        qT_sb = kvq.tile([128, NSLOT, NH, 128], bf16)
        for kvh in range(NKV):
            nc.vector.memset(v_sb[:, :, kvh * 129 + 128:kvh * 129 + 129], 1.0)

        # ---- Phase 1a-K: K projection (+RoPE) over all S kv tokens ----
        with ExitStack() as ph:
            xp = ph.enter_context(tc.tile_pool(name="x_k", bufs=2))
            wkp = ph.enter_context(tc.tile_pool(name="w_k", bufs=1))
            csp = ph.enter_context(tc.tile_pool(name="cs_k", bufs=2))
            psk = ph.enter_context(tc.tile_pool(name="ps_k", bufs=3, space="PSUM"))
            rp = ph.enter_context(tc.tile_pool(name="rope_kv", bufs=3))
            wkc = wkp.tile([128, 16, KVD], bf16)
            nc.sync.dma_start(out=wkc, in_=wk_r)
            for c in range(S // 512):
                cc = slice(c * 512, (c + 1) * 512)
                xc = xp.tile([128, 16, 512], bf16)
                nc.sync.dma_start(out=xc, in_=xT_r[:, :, cc])
                ck = csp.tile([128, 512], f32, tag="ck")
                nc.sync.dma_start(out=ck, in_=cosk[:, cc])
                sk = csp.tile([128, 512], f32, tag="sk")
                nc.sync.dma_start(out=sk, in_=sink[:, cc])
                for kvh in range(NKV):
                    ps = psk.tile([128, 512], f32)
                    for kt in range(16):
                        nc.tensor.matmul(
                            ps,
                            wkc[:, kt, kvh * 128:(kvh + 1) * 128],
                            xc[:, kt, :],
                            start=(kt == 0),
                            stop=(kt == 15),
                        )
                    rope(rp, ps, ck, sk, kT_sb[:, kvh, cc])

        # ---- Phase 1a-V: V projection (xT re-streamed) ----
        with ExitStack() as ph:
            xp = ph.enter_context(tc.tile_pool(name="x_v", bufs=2))
            wvp = ph.enter_context(tc.tile_pool(name="w_v", bufs=1))
            psv = ph.enter_context(tc.tile_pool(name="ps_v", bufs=3, space="PSUM"))
            wvc = wvp.tile([128, 16, KVD], bf16)
            nc.sync.dma_start(out=wvc, in_=wv_r)
            for c in range(S // 512):
                cc = slice(c * 512, (c + 1) * 512)
                xc = xp.tile([128, 16, 512], bf16)
                nc.sync.dma_start(out=xc, in_=xT_r[:, :, cc])
                for tbl in range(4):
                    tb = c * 4 + tbl
                    for dc in range(2):
                        ps = psv.tile([128, 512], f32)
                        for kt in range(16):
                            nc.tensor.matmul(
                                ps,
                                xc[:, kt, tbl * 128:(tbl + 1) * 128],
                                wvc[:, kt, dc * 512:(dc + 1) * 512],
                                start=(kt == 0),
                                stop=(kt == 15),
                            )
                        for j in range(4):
                            kvh = dc * 4 + j
                            dst = v_sb[:, tb, kvh * 129:kvh * 129 + 128]
                            src = ps[:, j * 128:(j + 1) * 128]
                            if j % 2 == 0:
                                nc.vector.tensor_copy(dst, src)
                            else:
                                nc.scalar.copy(dst, src)

        # ---- Phase 1b: Q projection (+RoPE, 1/sqrt(HD) folded into tables) ----
        with ExitStack() as ph:
            xp = ph.enter_context(tc.tile_pool(name="x_q", bufs=1))
            wp = ph.enter_context(tc.tile_pool(name="w_q", bufs=2))
            csp = ph.enter_context(tc.tile_pool(name="cs_q", bufs=1))
            psq = ph.enter_context(tc.tile_pool(name="ps_q", bufs=3, space="PSUM"))
            rp = ph.enter_context(tc.tile_pool(name="rope_q", bufs=3))
            xq = xp.tile([128, 16, SQ], bf16)
            nc.sync.dma_start(out=xq, in_=xqT_r)
            cq = csp.tile([128, SQ], f32)
            nc.sync.dma_start(out=cq, in_=cosq)
            sq = csp.tile([128, SQ], f32)
            nc.sync.dma_start(out=sq, in_=sinq)
            for quarter in range(4):
                wqc = wp.tile([128, 16, 512], bf16)
                nc.sync.dma_start(
                    out=wqc, in_=wq_r[:, :, quarter * 512:(quarter + 1) * 512]
                )
                for hl in range(4):
                    h = quarter * 4 + hl
                    for t in range(2):
                        tt = slice(t * 512, (t + 1) * 512)
                        ps = psq.tile([128, 512], f32)
                        for kt in range(16):
                            nc.tensor.matmul(
                                ps,
                                wqc[:, kt, hl * 128:(hl + 1) * 128],
                                xq[:, kt, tt],
                                start=(kt == 0),
                                stop=(kt == 15),
                            )
                        rope(
                            rp, ps, cq[:, tt], sq[:, tt],
                            qT_sb[:, 4 * t:4 * t + 4, h, :],
                        )

        # ---- Phase 2: attention (AttnOut^T bounced to DRAM scratch) ----
        # Scores are computed transposed (S^T[key, q]) so exp writes P^T
        # directly - no P transposes or PSUM->SBUF copies. Both heads of a GQA
        # pair share the stationary K^T block and are batched into one PSUM
        # tile / one exp. A ones-column appended to V makes the AV matmul
        # accumulate the softmax denominator for free; the division lands on
        # the small [q, d+1] AV output as a per-partition tensor_scalar.
        with ExitStack() as ph:
            pssc = ph.enter_context(tc.tile_pool(name="ps_sT", bufs=3, space="PSUM"))
            psot = ph.enter_context(tc.tile_pool(name="ps_ot", bufs=2, space="PSUM"))
            psoT = ph.enter_context(tc.tile_pool(name="ps_oT", bufs=1, space="PSUM"))
            ptp = ph.enter_context(tc.tile_pool(name="pT", bufs=14))
            stat = ph.enter_context(tc.tile_pool(name="stat", bufs=8))
            onp = ph.enter_context(tc.tile_pool(name="o_norm", bufs=6))
            ost = ph.enter_context(tc.tile_pool(name="ot_stage", bufs=8))
            for s in range(NSLOT):
                nkb = 2 * (s + 1)
                qs = slice(s * 128, (s + 1) * 128)
                for kvh in range(NKV):
                    h0 = 2 * kvh
                    ots_pair = [
                        psot.tile(
                            [128, HD + 1], f32,
                            tag=f"ot{j}", name=f"ot{j}_{s}_{kvh}",
                        )
                        for j in range(2)
                    ]
                    # kv-blocks processed two at a time: both scores land in
                    # one full-bank [128, 2, 256] PSUM tile -> one exp, and
                    # the causal mask (always the last two blocks) is one TT.
                    for p in range(s + 1):
                        sT = pssc.tile([128, 2, 256], f32)
                        for kl in range(2):
                            nc.tensor.matmul(
                                sT[:, kl, :],
                                kT_sb[:, kvh, (2 * p + kl) * 128:(2 * p + kl + 1) * 128],
                                qT_sb[:, s, h0:h0 + 2, :],
                                start=True,
                                stop=True,
                            )
                        if p == s:
                            nc.vector.tensor_tensor(sT, sT, mask_sb, ADD)
                        pts = ptp.tile([128, 2, 256], bf16)
                        nc.scalar.activation(pts, sT, EXP)
                        for kl in range(2):
                            kb = 2 * p + kl
                            for j in range(2):
                                nc.tensor.matmul(
                                    ots_pair[j],
                                    pts[:, kl, j * 128:(j + 1) * 128],
                                    v_sb[:, kb, kvh * 129:kvh * 129 + 129],
                                    start=(kb == 0),
                                    stop=(kb == nkb - 1),
                                )
                    for j in range(2):
                        ot = ots_pair[j]
                        rec = stat.tile([128, 1], f32, tag="rec")
                        nc.vector.reciprocal(rec, ot[:, HD:HD + 1])
                        onorm = onp.tile([128, 128], bf16)
                        nc.vector.tensor_scalar_mul(onorm, ot[:, 0:HD], rec)
                        oT = psoT.tile([128, 128], bf16)
                        nc.tensor.transpose(oT, onorm, ident)
                        ots = ost.tile([128, 128], bf16)
                        nc.vector.tensor_copy(ots, oT)
                        nc.sync.dma_start(
                            out=attnT_dram[(h0 + j) * 128:(h0 + j + 1) * 128, qs],
                            in_=ots,
                        )

        # ---- Phase 3: o_proj (attnT streamed back from DRAM) ----
        with ExitStack() as ph:
            wp = ph.enter_context(tc.tile_pool(name="w_o", bufs=2))
            ap_ = ph.enter_context(tc.tile_pool(name="attn_in", bufs=1))
            pso = ph.enter_context(tc.tile_pool(name="ps_o", bufs=3, space="PSUM"))
            op = ph.enter_context(tc.tile_pool(name="o_stage", bufs=4))
            ats = []
            for tb in range(NSLOT):
                at = ap_.tile([128, 16, 128], bf16, tag=f"at{tb}")
                nc.sync.dma_start(
                    out=at, in_=attnT_r[:, :, tb * 128:(tb + 1) * 128]
                )
                ats.append(at)
            for hc in range(4):
                woc = wp.tile([128, 16, 512], bf16)
                nc.sync.dma_start(out=woc, in_=wo_r[:, :, hc * 512:(hc + 1) * 512])
                for tb in range(NSLOT):
                    at = ats[tb]
                    ps = pso.tile([128, 512], f32)
                    for kt in range(16):
                        nc.tensor.matmul(
                            ps,
                            at[:, kt, :],
                            woc[:, kt, :],
                            start=(kt == 0),
                            stop=(kt == 15),
                        )
                    st = op.tile([128, 512], f32)
                    if tb % 2 == 0:
                        nc.vector.tensor_copy(st, ps)
                    else:
                        nc.scalar.copy(st, ps)
                    nc.sync.dma_start(
                        out=out[tb * 128:(tb + 1) * 128, hc * 512:(hc + 1) * 512],
                        in_=st,
                    )
    nc.compile()
    return nc


@functools.lru_cache(maxsize=1)
def _program():
    return _build_program()


def _host_prep(x, cos, sin, Wq, Wk, Wv, Wo):
    x = np.asarray(x, dtype=np.float32)
    cos = np.asarray(cos, dtype=np.float32)
    sin = np.asarray(sin, dtype=np.float32)
    scale = 1.0 / math.sqrt(HD)

    wq_b = np.asarray(Wq, dtype=np.float32).astype(BF16)
    wk_b = np.asarray(Wk, dtype=np.float32).astype(BF16)
    wv_b = np.asarray(Wv, dtype=np.float32).astype(BF16)
    wo_b = np.asarray(Wo, dtype=np.float32).astype(BF16)

    cosT = np.ascontiguousarray(cos.T)            # [HD, S]
    sinT = np.ascontiguousarray(sin.T)
    sinT_eff = sinT.copy()
    sinT_eff[: HD // 2] *= -1.0                   # fold rotate_half signs

    qcols = {}
    for g in range(2):
        qcols[g] = np.concatenate(
            [np.arange(128 * (2 * s + g), 128 * (2 * s + g) + 128) for s in range(NSLOT)]
        )

    masks, cosqs, sinqs = {}, {}, {}
    ii = np.arange(128)[:, None]
    jj = np.arange(256)[None, :]
    for g in range(2):
        # mask[q, j] over the last 256 keys of a slot; transposed to [key, q]
        # per 128-block and tiled x2 along q for the GQA-pair-batched scores.
        m = np.where(jj <= 128 * g + ii, 0.0, NEG).astype(np.float32)
        mT = np.ascontiguousarray(m.T)  # [256 key, 128 q]
        masks[g] = np.stack(
            [np.concatenate([mT[:128], mT[:128]], 1),
             np.concatenate([mT[128:], mT[128:]], 1)],
            axis=1,
        )  # [128 key, 2 block, 256 (q tiled x2)]
        cosqs[g] = np.ascontiguousarray(cosT[:, qcols[g]] * scale)
        sinqs[g] = np.ascontiguousarray(sinT_eff[:, qcols[g]] * scale)

    in_maps = []
    for c in range(NCORES):
        b, g = c // 2, c % 2
        xT_b = np.ascontiguousarray(x[b].T).astype(BF16)   # [H, S]
        in_maps.append(
            {
                "xT": xT_b,
                "xqT": np.ascontiguousarray(xT_b[:, qcols[g]]),
                "wq": wq_b,
                "wk": wk_b,
                "wv": wv_b,
                "wo": wo_b,
                "cosq": cosqs[g],
                "sinq": sinqs[g],
                "cosk": cosT,
                "sink": sinT_eff,
                "maskT": masks[g],
            }
        )
    return in_maps, qcols


def _assemble(results, qcols):
    full = np.empty((B, S, H), dtype=np.float32)
    for c in range(NCORES):
        b, g = c // 2, c % 2
        full[b].reshape(16, 128, H)[
            [2 * s + g for s in range(NSLOT)]
        ] = results[c]["out"].reshape(NSLOT, 128, H)
    return full


LAST_RESULTS = None


def kernel(x, cos, sin, Wq, Wk, Wv, Wo, _trace=False):
    global LAST_RESULTS
    from concourse.bass_utils import run_bass_kernel_spmd

    in_maps, qcols = _host_prep(x, cos, sin, Wq, Wk, Wv, Wo)
    res = run_bass_kernel_spmd(
        _program(),
        in_maps,
        core_ids=list(range(NCORES)),
        trace=_trace,
        trace_cores=list(range(NCORES)) if _trace else None,
    )
    LAST_RESULTS = res
    return _assemble(res.results, qcols)

